# revision 39
# baseline (speedup 1.0000x reference)
"""TRN2 Bass kernel for nn_AttributeClassifierHeaders (dense per-head MLP).

Computes y[b, a] = sigmoid(gelu(x @ W1[a] + b1[a]) . W2[a] + b2[a]) for 40
heads, sharded 5 heads per NeuronCore across 8 cores (head-parallel: each
head's weights are independent; x is replicated).

Stage-1 runs on the PE in fp8(e4m3) with perf_mode=DoubleRow: x and W1 are
quantized host-side (scales SX/SW keep values in e4m3's normal range; the
gelu activation un-scales via its fused `scale`), the contraction runs as 8
double-chunks of 256 (two fp8 weights per PE cell => 2x bf16 throughput).
End-to-end rel err vs the fp32 reference is ~1.3e-2, inside the 2e-2 gate.

Everything is SBUF-resident: x (8.4 MiB as 16 fp8 chunk tiles) AND all 5
heads' W1 (10.5 MiB as 40 fp8 tiles) are DMAed once at program start, so
the compute loop has no input DMA at all. Layout per 256-chunk c: slot
(p, i) holds contraction index d = c*256 + i*128 + p, identically for the
stationary W1 tile [128, 2, 128] and the moving x tile [128, 2, 512].

gelu+bias+descale fuse on ScalarE out of PSUM (bf16 out). The per-head dot
product runs on the otherwise-idle DVE (acc += ht * w2col per hid-tile m,
bf16), then a single ones-vector matmul per (head, batch-chunk) reduces
the 128 partitions into PSUM. The reduce matmul and sigmoid (fused bias
b2, tanh form so it shares gelu's activation-table set) are deferred and
popped one per stage-1 group during the FOLLOWING head's compute, so the
in-order PE queue never waits on the DVE accumulation. Outputs stage into
a per-head SBUF row and ship as one background DMA per head on the
otherwise-idle gpsimd queue, keeping the ACT queue (which evacuates PSUM)
free of per-chunk DMA triggers (~667 ns of sequencer time each).

HW-measured (repeat-differenced): 690.9 us/iter, rel err 1.317e-2 vs the
fp32 reference (gate 2e-2). PE-stream floor for the 2560 DoubleRow MMs is
~531 us (207.5 ns/MM probed in isolation); known-good overheads: PSUM
evacuation coupling ~19 ns/MM, stage-2 epilogues ~1.5 us each. Attempts
that did NOT help on HW: LDWEIGHTS dedup (hidden by PE pull-ahead),
wider stationary sharing (pairs->quads), n-outer last head.
"""
import os
import sys
from contextlib import ExitStack

import numpy as np
import ml_dtypes

for _p in ("/root/.axon_site/_ro/trn_rl_repo", "/opt/trn_rl_repo"):
    if os.path.isdir(_p) and _p not in sys.path:
        sys.path.append(_p)

import jax  # noqa: E402
from jax.sharding import Mesh, PartitionSpec, NamedSharding  # noqa: E402
from jax.experimental.shard_map import shard_map  # noqa: E402

import concourse.bacc as bacc  # noqa: E402
import concourse.tile as tile  # noqa: E402
from concourse import mybir, bass2jax  # noqa: E402

F32 = mybir.dt.float32
F8 = mybir.dt.float8e4
BF = mybir.dt.bfloat16
AF = mybir.ActivationFunctionType
DR = mybir.MatmulPerfMode.DoubleRow
ALU = mybir.AluOpType

# problem shape (hardcoded; see module docstring)
B, D, A, H = 4096, 2048, 40, 1024
NCORES = 8
APC = A // NCORES        # 5 heads per core
KT = D // 128            # 16 contraction 128-tiles
KC = KT // 2             # 8 DoubleRow 256-chunks
MT = H // 128            # 8 hid tiles
NQ = 2                   # batch halves
QTR = B // NQ            # 2048
NCH = QTR // 512         # 512-wide chunks per half

SX = 16.0                # x fp8 scale (|x|<~6 -> <96, e4m3 normal range)
SW = 4096.0              # W1 fp8 scale (|W1|<=0.0221 -> <=90.5)
INV = 1.0 / (SX * SW)    # descale fused into the gelu activation

E4NP = ml_dtypes.float8_e4m3   # == mybir.dt.np(float8e4): TRN variant, max 240
BFNP = ml_dtypes.bfloat16


def build_program(repeat: int = 0):
    nc = bacc.Bacc("TRN2", target_bir_lowering=False, debug=False)
    x8_d = nc.dram_tensor("x8", [NQ * KC, 128, 2, QTR], F8,
                          kind="ExternalInput").ap()
    w1_d = nc.dram_tensor("w1p", [APC, MT, 128, KT, 128], F8,
                          kind="ExternalInput").ap()
    b1_d = nc.dram_tensor("b1p", [APC, 128, MT], F32, kind="ExternalInput").ap()
    w2_d = nc.dram_tensor("w2p", [APC, 128, MT], F32, kind="ExternalInput").ap()
    b2_d = nc.dram_tensor("b2p", [1, APC], F32, kind="ExternalInput").ap()
    y_d = nc.dram_tensor("y", [APC, B], F32, kind="ExternalOutput").ap()

    with tile.TileContext(nc) as tc, ExitStack() as ctx:
        const = ctx.enter_context(tc.tile_pool(name="const", bufs=1))
        xres = ctx.enter_context(tc.tile_pool(name="xres", bufs=1))
        wres = ctx.enter_context(tc.tile_pool(name="wres", bufs=1))
        sp = ctx.enter_context(tc.tile_pool(name="sp", bufs=2))
        yp = ctx.enter_context(tc.tile_pool(name="yp", bufs=2))
        hp = ctx.enter_context(tc.tile_pool(name="hp", bufs=6))
        ap_ = ctx.enter_context(tc.tile_pool(name="accp", bufs=2))
        tp = ctx.enter_context(tc.tile_pool(name="tmpp", bufs=4))
        ps1 = ctx.enter_context(tc.tile_pool(name="ps1", bufs=6, space="PSUM"))
        ps2 = ctx.enter_context(tc.tile_pool(name="ps2", bufs=2, space="PSUM"))

        # ---- one-time setup: constants + ALL of x and W1 resident in SBUF.
        # Inside the repeat loop these are only read, so iterations 2..R have
        # no input-DMA waits at all.
        b1t = const.tile([128, APC * MT], F32, tag="b1t")
        w2t = const.tile([128, APC * MT], F32, tag="w2t")
        b2t = const.tile([1, APC], F32, tag="b2t")
        ones = const.tile([128, 1], BF, tag="ones")
        nc.vector.memset(ones[:], 1.0)
        for a in range(APC):
            nc.sync.dma_start(b1t[:, a * MT:(a + 1) * MT], b1_d[a])
            nc.sync.dma_start(w2t[:, a * MT:(a + 1) * MT], w2_d[a])
        nc.sync.dma_start(b2t[:], b2_d[:])
        xh = [[xres.tile([128, 2, QTR], F8, tag=f"x{hf}c{c}", name=f"x{hf}c{c}")
               for c in range(KC)] for hf in range(NQ)]
        wt = [[wres.tile([128, KT, 128], F8, tag=f"w{a}m{m}", name=f"w{a}m{m}")
               for m in range(MT)] for a in range(APC)]
        # first tiles land first (single-shot ramp); x rides the gpsimd
        # queue in interleaved half order so both halves fill together,
        # W streams on sync.
        nc.sync.dma_start(wt[0][0][:], w1_d[0, 0])
        for c in range(KC):
            for hf in range(NQ):
                nc.gpsimd.dma_start(xh[hf][c][:], x8_d[hf * KC + c])
        for a in range(APC):
            for m in range(MT):
                if a == 0 and m == 0:
                    continue
                nc.sync.dma_start(wt[a][m][:], w1_d[a, m])

        def compute():
            # per-(head, half, chunk) epilogues (ones-reduce matmul +
            # sigmoid + output DMA) are deferred and popped one per stage-1
            # group during the FOLLOWING head's groups, so the in-order PE
            # queue never waits on the DVE accumulation that produces their
            # input (acc pool bufs=2 keeps the previous head's acc alive).
            pending = []
            acc = {}
            # per-head staging row for outputs: the 8 sigmoid chunks land
            # here via DVE, then ONE background DMA per head on the
            # otherwise-idle gpsimd queue ships them.
            ybuf = {}

            def tail(a, hf, n, m, pt):
                ht = hp.tile([128, 512], BF, tag="ht", name="ht")
                nc.scalar.activation(ht[:], pt[:], AF.Gelu,
                                     bias=b1t[:, a * MT + m:a * MT + m + 1],
                                     scale=INV)
                w2col = w2t[:, a * MT + m:a * MT + m + 1]
                if m == 0:
                    acc_t = ap_.tile([128, 512], BF, tag=f"acc{hf}{n}",
                                     name="acc_t")
                    acc[(hf, n)] = acc_t
                    nc.vector.tensor_scalar_mul(acc_t[:], ht[:], w2col)
                else:
                    tmp = tp.tile([128, 512], BF, tag="tmp", name="tmp")
                    nc.vector.tensor_scalar_mul(tmp[:], ht[:], w2col)
                    nc.vector.tensor_tensor(acc[(hf, n)][:], acc[(hf, n)][:],
                                            tmp[:], ALU.add)
                if m == MT - 1:
                    acc_t = acc[(hf, n)]

                    def epilogue(a=a, hf=hf, n=n, acc_t=acc_t):
                        psy = ps2.tile([1, 512], F32, tag="psy", name="psy")
                        nc.tensor.matmul(psy[:], ones[:], acc_t[:],
                                         start=True, stop=True,
                                         skip_group_check=True)
                        # sigmoid(z) = 0.5*tanh(z/2) + 0.5: tanh shares
                        # gelu's activation-table set, so no ACT_TABLE_LOAD
                        # ping-pong; b2p is pre-halved host-side.
                        stg = sp.tile([1, 512], F32, tag="stg", name="stg")
                        nc.scalar.activation(stg[:], psy[:], AF.Tanh,
                                             bias=b2t[0:1, a:a + 1], scale=0.5)
                        off = hf * QTR + n * 512
                        nc.vector.tensor_scalar(ybuf[a][0:1, off:off + 512],
                                                stg[:], 0.5, 0.5,
                                                ALU.mult, ALU.add)
                        if hf == NQ - 1 and n == NCH - 1:
                            nc.gpsimd.dma_start(y_d[a:a + 1, :], ybuf[a][:])
                    pending.append(epilogue)

            for a in range(APC):
                ybuf[a] = yp.tile([1, B], F32, tag="ybuf", name=f"yb{a}")
                for m in range(MT):
                    for hf in range(NQ):
                        pts = [ps1.tile([128, 512], F32, tag="ps1",
                                        name=f"pt{j}") for j in range(NCH)]
                        for c in range(KC):
                            for j in range(NCH):
                                nc.tensor.matmul(
                                    pts[j][:], wt[a][m][:, 2 * c:2 * c + 2, :],
                                    xh[hf][c][:, :, j * 512:(j + 1) * 512],
                                    start=(c == 0), stop=(c == KC - 1),
                                    perf_mode=DR)
                        if pending:
                            pending.pop(0)()
                        for j in range(NCH):
                            tail(a, hf, j, m, pts[j])
            while pending:
                pending.pop(0)()

        if repeat and repeat > 1:
            with tc.For_i(0, repeat, 1):
                compute()
        else:
            compute()
    # dedup disabled: the LDW-per-MM rhythm measured FASTER (probe replica
    # without dedup runs 561us; v2-era A/B also favored no-dedup slightly)
    # _dedup_ldweights(nc)
    nc.compile()
    return nc


def _thin_mm_sem_updates(nc):
    """Strip per-matmul semaphore increments that no waiter's threshold
    lands on, renumbering the remaining thresholds. The tile framework
    counts EVERY matmul completion on one counter sem while only chain-stop
    counts are ever waited on; each increment costs the PE sequencer
    ~20 ns (2600/iter). Keeps increments whose cumulative count equals
    some wait threshold, rewrites wait immediates to ranks among kept
    increments, and patches the loop prime/reset (+-total) constants.
    Bails out (no-op) on any unexpected encoding."""
    for f in nc.m.functions:
        # locate the counter sem: the one incremented by matmuls
        from collections import Counter as _C
        inc_by_sem = _C()
        for blk in f.blocks:
            for inst in blk.instructions:
                si = getattr(inst, "sync_info", None)
                if si is None or not isinstance(inst, mybir.InstMatmult):
                    continue
                for u in si.on_update:
                    if str(u.update_mode) == "sem-inc":
                        inc_by_sem[u.id] += 1
        if not inc_by_sem:
            continue
        sem_id, n_inc = inc_by_sem.most_common(1)[0]
        waits, consts, mm_updates = [], [], []
        ok = True
        for blk in f.blocks:
            pos = 0
            for inst in blk.instructions:
                si = getattr(inst, "sync_info", None)
                if si is None:
                    continue
                for w in si.on_wait:
                    if w.id == sem_id:
                        if not w.uses_immediate or str(w.wait_mode) != "sem-ge-imm":
                            ok = False
                        waits.append(w)
                for u in si.on_update:
                    if u.id != sem_id:
                        continue
                    mode = str(u.update_mode)
                    if mode == "sem-inc" and isinstance(inst, mybir.InstMatmult):
                        pos += 1
                        mm_updates.append((inst, u, pos))
                    elif mode in ("sem-add-imm", "sem-sub-imm") \
                            and u.update_value == n_inc:
                        consts.append(u)
                    else:
                        ok = False
        if not ok or len(mm_updates) != n_inc:
            continue
        thresholds = set(w.wait_value for w in waits)
        if not thresholds.issubset(set(range(1, n_inc + 1))):
            continue
        thresholds.add(n_inc)  # keep the final count for the loop reset
        rank, kept = {}, 0
        for _, _, pos in mm_updates:
            if pos in thresholds:
                kept += 1
                rank[pos] = kept
        for inst, u, pos in mm_updates:
            if pos not in thresholds:
                inst.sync_info.on_update = [
                    x for x in inst.sync_info.on_update if x is not u]
        for w in waits:
            w.wait_value = rank[w.wait_value]
        for u in consts:
            u.update_value = kept


def _ap_key(ap):
    memref = getattr(ap, "memref", None)
    return (getattr(ap, "offset", None),
            tuple(tuple(d) for d in getattr(ap, "ap", ())),
            getattr(memref, "name", memref),
            getattr(ap, "dtype", None))


def _dedup_ldweights(nc):
    """Drop InstLdweights whose weights AP matches the previous load on the
    PE queue (tile_legalize emits one per matmul unconditionally; the PE
    array keeps its stationary operand across matmuls). Runs post-scheduler
    / pre-compile; only drops loads with no semaphore waits/updates, and
    any other PE-queue instruction resets the tracked state."""
    dropped = 0
    for f in nc.m.functions:
        for blk in f.blocks:
            last_key = None
            out = []
            for inst in blk.instructions:
                if isinstance(inst, mybir.InstLdweights):
                    si = inst.sync_info
                    clean = si is None or (not si.on_wait and not si.on_update)
                    key = (_ap_key(inst.ins[0]), inst.perf_mode,
                           inst.is_transpose, inst.tile_position,
                           inst.tile_size)
                    if clean and key == last_key:
                        dropped += 1
                        continue
                    if clean:
                        last_key = key
                    else:
                        last_key = None
                elif isinstance(inst, mybir.InstMatmult):
                    pass  # matmul does not disturb the loaded weights
                elif getattr(inst, "engine", None) == mybir.EngineType.PE:
                    last_key = None
                out.append(inst)
            blk.instructions = out
    return dropped


class _Runner:
    """jit-once PJRT runner for a prebuilt Bass program (8-core SPMD)."""

    def __init__(self, nc, n_cores):
        bass2jax.install_neuronx_cc_hook()
        self.nc = nc
        self.n_cores = n_cores
        in_names, out_names, out_avals, zero_outs = [], [], [], []
        for alloc in nc.m.functions[0].allocations:
            if not isinstance(alloc, mybir.MemoryLocationSet):
                continue
            name = alloc.memorylocations[0].name
            if alloc.kind == "ExternalInput":
                in_names.append(name)
            elif alloc.kind == "ExternalOutput":
                shape = tuple(alloc.tensor_shape)
                dtype = mybir.dt.np(alloc.dtype)
                out_names.append(name)
                out_avals.append(jax.core.ShapedArray(shape, dtype))
                zero_outs.append(np.zeros(shape, dtype))
        partition_name = (nc.partition_id_tensor.name
                          if nc.partition_id_tensor else None)
        if partition_name is not None and partition_name in in_names:
            in_names.remove(partition_name)
        self.in_names = in_names
        self.out_names = out_names
        self.zero_outs = zero_outs
        n_params = len(in_names)
        n_outs = len(out_avals)
        all_in_names = list(in_names) + list(out_names)
        if partition_name is not None:
            all_in_names.append(partition_name)
        donate = tuple(range(n_params, n_params + n_outs))

        def _body(*args):
            operands = list(args)
            if partition_name is not None:
                operands.append(bass2jax.partition_id_tensor())
            outs = bass2jax._bass_exec_p.bind(
                *operands,
                out_avals=tuple(out_avals),
                in_names=tuple(all_in_names),
                out_names=tuple(out_names),
                lowering_input_output_aliases=(),
                sim_require_finite=True,
                sim_require_nnan=True,
                nc=nc,
            )
            return tuple(outs)

        devices = jax.devices()[:n_cores]
        assert len(devices) == n_cores, f"need {n_cores} neuron cores"
        self.mesh = Mesh(np.asarray(devices), ("core",))
        in_specs = (PartitionSpec("core"),) * (n_params + n_outs)
        out_specs = (PartitionSpec("core"),) * n_outs
        self.fn = jax.jit(
            shard_map(_body, mesh=self.mesh, in_specs=in_specs,
                      out_specs=out_specs, check_rep=False),
            donate_argnums=donate, keep_unused=True,
        )
        self._dev_inputs = None

    def put_inputs(self, in_maps):
        sharding = NamedSharding(self.mesh, PartitionSpec("core"))
        self._dev_inputs = [
            jax.device_put(
                np.concatenate([np.asarray(m[name]) for m in in_maps], axis=0),
                sharding)
            for name in self.in_names
        ]

    def run(self):
        sharding = NamedSharding(self.mesh, PartitionSpec("core"))
        zouts = [jax.device_put(np.concatenate([z] * self.n_cores, axis=0),
                                sharding) for z in self.zero_outs]
        outs = self.fn(*self._dev_inputs, *zouts)
        jax.block_until_ready(outs)
        return outs

    def run_np(self):
        outs = self.run()
        res = []
        for c in range(self.n_cores):
            d = {}
            for i, name in enumerate(self.out_names):
                full = np.asarray(outs[i])
                per = full.shape[0] // self.n_cores
                d[name] = full[c * per:(c + 1) * per]
            res.append(d)
        return res


_CACHE = {}


def _get_runner(repeat=0):
    if repeat not in _CACHE:
        _CACHE[repeat] = _Runner(build_program(repeat), NCORES)
    return _CACHE[repeat]


def _q8(a, scale):
    return np.clip(np.asarray(a, np.float32) * scale,
                   -240.0, 240.0).astype(E4NP)


def make_in_maps(x, W1, b1, W2, b2):
    x = np.asarray(x, dtype=np.float32)
    W1 = np.asarray(W1, dtype=np.float32)
    b1 = np.asarray(b1, dtype=np.float32)
    W2 = np.asarray(W2, dtype=np.float32)
    b2 = np.asarray(b2, dtype=np.float32)
    # x8[(hf*KC+c), p, i*QTR+n] = fp8(SX * x[hf*QTR+n, c*256+i*128+p]):
    # per-chunk contiguous 512 KiB blocks matching the [128, 2, QTR] tiles
    xq = _q8(x, SX).T
    x8 = np.ascontiguousarray(
        xq.reshape(KC, 2, 128, NQ, QTR).transpose(3, 0, 2, 1, 4)
        .reshape(NQ * KC, 128, 2, QTR))
    # W1p[a, m, p, ks, c] = fp8(SW * W1[a, ks*128+p, m*128+c]) (per-(a,m)
    # contiguous 256 KiB block; DoubleRow pairs are ks slots (2c, 2c+1))
    W1p = np.ascontiguousarray(
        _q8(W1, SW).reshape(A, KT, 128, MT, 128).transpose(0, 3, 2, 1, 4)
        .reshape(A, MT, 128, KT, 128))
    b1p = np.ascontiguousarray(b1.reshape(A, MT, 128).transpose(0, 2, 1))
    W2p = np.ascontiguousarray(W2.reshape(A, MT, 128).transpose(0, 2, 1))
    b2p = np.ascontiguousarray(0.5 * b2.reshape(1, A))  # tanh-form sigmoid
    in_maps = []
    for c in range(NCORES):
        s = slice(c * APC, (c + 1) * APC)
        in_maps.append({"x8": x8, "w1p": W1p[s], "b1p": b1p[s],
                        "w2p": W2p[s], "b2p": b2p[:, s]})
    return in_maps


def kernel(x, W1, b1, W2, b2):
    in_maps = make_in_maps(x, W1, b1, W2, b2)
    r = _get_runner(0)
    r.put_inputs(in_maps)
    outs = r.run_np()
    y = np.concatenate([outs[c]["y"] for c in range(NCORES)], axis=0)
    return np.ascontiguousarray(y.T).astype(np.float32)
